# revision 14
# baseline (speedup 1.0000x reference)
"""Distributed GCN (2x GCNConv + Linear) on 8 Trainium2 NeuronCores via Bass/Tile.

Algorithm (matches the PyG-style reference):
  h1 = relu(gcnconv(x, W1, b1, mask1));  h2 = relu(gcnconv(h1, W2, b2, mask2))
  out = h2 @ Wl + bl
where gcnconv(x, W, b, keep) with self-loops:
  h = x @ W;  deg = segsum(keep, dst) + 1;  dis = rsqrt(deg)
  out = segsum(h[src] * (keep * dis[src] * dis[dst]), dst) + h * dis^2 + b

Distribution/schedule (v2 — gather-prep pipelined):
  * Layer 1: every core computes the FULL H1 = X@W1 (replicated, bf16) and
    writes it to local DRAM in global node order — no AllGather for layer 1.
    A small per-core pass also computes the core's own-shard H1 into SBUF for
    the self-loop term.
  * Edges are partitioned by dst core; per (dst-tile-group g, src-section s)
    they form a gather "run" (padded to the max block count over cores so the
    SPMD program is shape-uniform; edges sorted by (dst tile, src) for HBM
    locality).  dma_gather descriptor generation runs ahead with
    prepare_only=True (queue = src section, two groups of lookahead);
    trigger_dma(count=None) fires a queue's pending preps once that
    section's H rows are resident (layer 1: local stores; layer 2:
    AllGather of that section).  This keeps GPSIMD descriptor generation
    off the critical path of the collectives and data stores.
  * Aggregation: per dst tile, PSUM accumulates G_blk^T @ M_blk where M_blk
    is built on device by one VectorE tensor_scalar per block:
    M[p, d] = (iota[d] == dl[p]) * cf[p], from tiny per-edge dl/cf streams
    (dst-slot + normalization coef).  Self-loop blocks use the own-shard H
    tile scaled by dis^2 against an identity.  ReLU+bias runs on ScalarE in
    the transposed layout; the next layer's H-matmul follows per tile.
  * Layer 2 H2 tiles are stored to a local shard buffer; a 4-section
    AllGather (interleaved with the per-section triggers) makes them
    available for the layer-2 gathers.

Host-side numpy does graph preprocessing only (edge partitioning, padding,
degree/normalization scalars, index/dl/cf layout); all O(N*F) / O(E*F) float
work runs on the NeuronCores.
"""

import numpy as np
import ml_dtypes

import concourse.bass as bass
import concourse.bacc as bacc
import concourse.tile as tile
import concourse.mybir as mybir
from concourse.bass_utils import run_bass_kernel_spmd
from concourse.tile_sem_assignment import PROC_NAME_TO_IDX

N_SWDGE_LANES = 8   # tile's NUM_SWDGE_GLOBAL_SEMS (DMASW proc lanes)
PREP_TRIGGER = False  # False: immediate gathers (AllGathers hoisted before them)

P = 128
N_CORES = 8

N_NODES = 50000
F_IN = 128
F_HID = 128
F_OUT = 64

N_PAD = 50176
SHARD = N_PAD // N_CORES            # 6272
TILES_PC = SHARD // P               # 49
N_SEC = 4
SEC_L1 = N_PAD // N_SEC             # 12544 (global row sections)
SEC_L2 = SHARD // N_SEC             # 1568  (per-shard row sections)
# dst-tile groups (local tile ranges per core)
GROUP_BOUNDS = [0, 7, 14, 21, 28, 35, 42, TILES_PC]
N_GROUPS = len(GROUP_BOUNDS) - 1
LOOK = 2                            # groups of gather-prep lookahead
DL_PAD = 255.0                      # sentinel dst-slot: matches no iota column


# ---------------------------------------------------------------------------
# Host-side preprocessing
# ---------------------------------------------------------------------------

class _Run:
    __slots__ = ("g", "s", "nblk", "col0", "rid")

    def __init__(self, g, s, nblk, col0, rid):
        self.g, self.s, self.nblk, self.col0, self.rid = g, s, nblk, col0, rid


class _LayerLayout:
    __slots__ = ("runs", "run_by_gs", "n_builds", "idx_cols", "tile_sched")

    def __init__(self):
        self.runs = []            # group-major emission order
        self.run_by_gs = {}       # (g,s) -> _Run
        self.n_builds = 0
        self.idx_cols = 0
        # local tile -> [(rid, b_local, build_col), ...]
        self.tile_sched = {}


def _prep_layer(src_k, dst_k, coef_k, li):
    """Shared layout + per-core idx/dl/cf arrays for one layer."""
    src_k = src_k.astype(np.int64)
    dst_k = dst_k.astype(np.int64)
    if li == 0:
        sec = src_k // SEC_L1
        gidx = src_k % SEC_L1
    else:
        c_s = src_k // SHARD
        r = src_k % SHARD
        sec = r // SEC_L2
        gidx = c_s * SEC_L2 + (r % SEC_L2)
    core = dst_k // SHARD
    lt = (dst_k % SHARD) // P
    dl = dst_k % P
    gid = np.searchsorted(GROUP_BOUNDS, lt, side="right") - 1
    cf = coef_k.astype(np.float32)

    # per (core, g, s): sorted edge arrays + per-tile start offsets
    per_run = {}
    cnt = np.zeros((N_CORES, N_GROUPS, N_SEC), dtype=np.int64)
    for c in range(N_CORES):
        mc = core == c
        for g in range(N_GROUPS):
            t_lo, t_hi = GROUP_BOUNDS[g], GROUP_BOUNDS[g + 1]
            mg = mc & (gid == g)
            for s in range(N_SEC):
                m = mg & (sec == s)
                lt_r, gx_r, dl_r, cf_r = lt[m], gidx[m], dl[m], cf[m]
                o = np.lexsort((gx_r, lt_r))
                lt_r, gx_r, dl_r, cf_r = lt_r[o], gx_r[o], dl_r[o], cf_r[o]
                starts = np.searchsorted(lt_r, np.arange(t_lo, t_hi + 1))
                per_run[(c, g, s)] = (gx_r, dl_r, cf_r, starts)
                cnt[c, g, s] = len(gx_r)

    lay = _LayerLayout()
    col0 = 0
    rid = 0
    for g in range(N_GROUPS):
        for s in range(N_SEC):
            nblk = int(-(-cnt[:, g, s].max() // P))
            r = _Run(g, s, nblk, col0, rid)
            lay.runs.append(r)
            lay.run_by_gs[(g, s)] = r
            col0 += nblk * P // 16
            rid += 1
    lay.idx_cols = col0

    # shared build schedule: per tile, per sec, block span = union over cores
    j = 0
    builds = []   # (g, s, t, b, j)
    for g in range(N_GROUPS):
        t_lo, t_hi = GROUP_BOUNDS[g], GROUP_BOUNDS[g + 1]
        for t in range(t_lo, t_hi):
            sched = []
            for s in range(N_SEC):
                r = lay.run_by_gs[(g, s)]
                b0, b1 = 10 ** 9, 0
                for c in range(N_CORES):
                    starts = per_run[(c, g, s)][3]
                    a, b = starts[t - t_lo], starts[t - t_lo + 1]
                    if b > a:
                        b0 = min(b0, a // P)
                        b1 = max(b1, -(-b // P))
                for bb in range(b0, b1):
                    sched.append((r.rid, bb, j))
                    builds.append((g, s, t, bb, j))
                    j += 1
            lay.tile_sched[t] = sched
    lay.n_builds = j

    # per-core arrays
    per_core = []
    for c in range(N_CORES):
        idx16 = np.zeros((max(lay.idx_cols, 1) * 16,), dtype=np.int16)
        dla = np.full((P, max(j, 1)), DL_PAD, dtype=np.float32)
        cfa = np.zeros((P, max(j, 1)), dtype=np.float32)
        for r in lay.runs:
            gx_r = per_run[(c, r.g, r.s)][0]
            sl = slice(r.col0 * 16, r.col0 * 16 + len(gx_r))
            idx16[sl] = gx_r.astype(np.int16)
        for (g, s, t, bb, jj) in builds:
            gx_r, dl_r, cf_r, starts = per_run[(c, g, s)]
            t_lo = GROUP_BOUNDS[g]
            a, b = int(starts[t - t_lo]), int(starts[t - t_lo + 1])
            lo = max(a, bb * P)
            hi = min(b, (bb + 1) * P)
            if lo < hi:
                rows = np.arange(lo, hi)
                dla[rows - bb * P, jj] = dl_r[rows]
                cfa[rows - bb * P, jj] = cf_r[rows]
        w = idx16.reshape(-1, 16).T
        idxw = np.ascontiguousarray(np.tile(w, (8, 1)))
        per_core.append({"idx": idxw, "dl": dla, "cf": cfa})
    return lay, per_core


def _prepare(x, edge_index, mask1, mask2, W1, b1, W2, b2, Wl, bl,
             n, n_pad):
    assert n_pad == N_PAD
    bf16 = ml_dtypes.bfloat16
    src = np.asarray(edge_index[0], dtype=np.int64)
    dst = np.asarray(edge_index[1], dtype=np.int64)

    layouts = []
    layer_data = []
    selfws = []
    for li, mask in enumerate((np.asarray(mask1), np.asarray(mask2))):
        keep = mask.astype(bool)
        ks, kd = src[keep], dst[keep]
        deg = np.bincount(kd, minlength=n).astype(np.float64) + 1.0
        dis = 1.0 / np.sqrt(deg)
        coef_k = (dis[ks] * dis[kd]).astype(np.float32)
        selfw = np.zeros((n_pad,), dtype=np.float32)
        selfw[:n] = (dis * dis).astype(np.float32)
        lay, pc = _prep_layer(ks, kd, coef_k, li)
        layouts.append(lay)
        layer_data.append(pc)
        selfws.append(selfw)

    xp = np.zeros((n_pad, F_IN), dtype=np.float32)
    xp[:n] = np.asarray(x, dtype=np.float32)
    xt_full = np.ascontiguousarray(xp.T.astype(bf16))

    ident = np.eye(P, dtype=np.float32)
    iota = np.broadcast_to(np.arange(P, dtype=np.float32), (P, P)).copy()

    in_maps = []
    for c in range(N_CORES):
        m = {
            "xt": xt_full,
            "xto": np.ascontiguousarray(
                xp[c * SHARD:(c + 1) * SHARD].T.astype(bf16)),
            "w1": np.asarray(W1, np.float32).astype(bf16),
            "w2": np.asarray(W2, np.float32).astype(bf16),
            "wl": np.asarray(Wl, np.float32).astype(bf16),
            "b1c": np.asarray(b1, np.float32).reshape(P, 1),
            "b2c": np.asarray(b2, np.float32).reshape(P, 1),
            "blbc": np.broadcast_to(np.asarray(bl, np.float32),
                                    (P, F_OUT)).copy(),
            "ident": ident.astype(bf16),
            "iota": iota.astype(bf16),
        }
        for li in (0, 1):
            d = layer_data[li][c]
            m[f"idx{li+1}"] = d["idx"]
            m[f"dl{li+1}"] = d["dl"]
            m[f"cf{li+1}"] = d["cf"]
            sw = selfws[li][c * SHARD:(c + 1) * SHARD]
            m[f"sw{li+1}"] = np.ascontiguousarray(
                sw.reshape(TILES_PC, P).T.astype(np.float32))
        in_maps.append(m)
    return layouts, in_maps


# ---------------------------------------------------------------------------
# Device program
# ---------------------------------------------------------------------------

def _build(layouts, n_pad):
    assert n_pad == N_PAD
    gdt = mybir.dt.bfloat16
    f32 = mybir.dt.float32

    nc = bacc.Bacc("TRN2", target_bir_lowering=False, debug=False,
                   num_swdge_queues=N_SEC)

    xt_d = nc.declare_dram_parameter("xt", [P, N_PAD], gdt, isOutput=False)
    xto_d = nc.declare_dram_parameter("xto", [P, SHARD], gdt, isOutput=False)
    w1_d = nc.declare_dram_parameter("w1", [P, F_HID], gdt, isOutput=False)
    w2_d = nc.declare_dram_parameter("w2", [P, F_HID], gdt, isOutput=False)
    wl_d = nc.declare_dram_parameter("wl", [P, F_OUT], gdt, isOutput=False)
    b1c_d = nc.declare_dram_parameter("b1c", [P, 1], f32, isOutput=False)
    b2c_d = nc.declare_dram_parameter("b2c", [P, 1], f32, isOutput=False)
    blbc_d = nc.declare_dram_parameter("blbc", [P, F_OUT], f32, isOutput=False)
    ident_d = nc.declare_dram_parameter("ident", [P, P], gdt, isOutput=False)
    iota_d = nc.declare_dram_parameter("iota", [P, P], gdt, isOutput=False)
    idx_d, dl_d, cf_d, sw_d = [], [], [], []
    for li, lay in enumerate(layouts):
        ic = max(lay.idx_cols, 1)
        nb = max(lay.n_builds, 1)
        idx_d.append(nc.declare_dram_parameter(
            f"idx{li+1}", [P, ic], mybir.dt.int16, isOutput=False))
        dl_d.append(nc.declare_dram_parameter(
            f"dl{li+1}", [P, nb], f32, isOutput=False))
        cf_d.append(nc.declare_dram_parameter(
            f"cf{li+1}", [P, nb], f32, isOutput=False))
        sw_d.append(nc.declare_dram_parameter(
            f"sw{li+1}", [P, TILES_PC], f32, isOutput=False))
    out_d = nc.declare_dram_parameter("out", [SHARD, F_OUT], f32, isOutput=True)

    h1_sec = nc.dram_tensor("h1_sec", [N_PAD, P], gdt)
    h2_shard = nc.dram_tensor("h2_shard", [SHARD, P], gdt)
    h2_sec = [nc.dram_tensor(f"h2_sec{s}", [N_CORES * SEC_L2, P], gdt,
                             addr_space="Shared") for s in range(N_SEC)]

    rg = [list(range(N_CORES))]
    relu = mybir.ActivationFunctionType.Relu
    copyf = mybir.ActivationFunctionType.Copy
    max_run_nb = max((r.nblk for lay in layouts for r in lay.runs), default=1)
    # phase-0 streaming: 14 chunks of 28 tiles (3584 nodes) each
    XCH = 14
    XCH_T = N_PAD // (XCH * P)      # 28

    # Per-DMASW-lane completion semaphores.  Tile's managed path pre-bumps
    # its DMASW lane sems at prep time (descriptor-write), so data consumers
    # wired to DMASW would not wait for the triggered DMA.  We pass our own
    # sem per prep (fixed after scheduling to match the prep's DMASW lane)
    # and retarget all DMASW waits to these sems post-scheduling.
    glane = [nc.alloc_semaphore(f"glane{j}") for j in range(N_SWDGE_LANES)]
    prep_ctr = [0]

    with tile.TileContext(nc) as tc:
        with (
            tc.tile_pool(name="consts", bufs=1) as cpool,
            tc.tile_pool(name="xs", bufs=2) as xpool,
            tc.tile_pool(name="hstage", bufs=2) as stpool,
            tc.tile_pool(name="gbuf", bufs=LOOK * N_SEC) as gpool,
            tc.tile_pool(name="mpool", bufs=6) as mpool,
            tc.tile_pool(name="spool", bufs=4) as spool,
            tc.tile_pool(name="opool", bufs=8) as opool,
            tc.tile_pool(name="aggp", bufs=4, space="PSUM") as aggpool,
            tc.tile_pool(name="hp", bufs=4, space="PSUM") as hpool,
        ):
            def load_const(dram, shape, dt):
                t = cpool.tile(shape, dt, tag=dram.name)
                nc.sync.dma_start(t[:], dram[:])
                return t

            xto_sb = load_const(xto_d, [P, SHARD], gdt)
            w1_sb = load_const(w1_d, [P, F_HID], gdt)
            w2_sb = load_const(w2_d, [P, F_HID], gdt)
            wl_sb = load_const(wl_d, [P, F_OUT], gdt)
            b1c_sb = load_const(b1c_d, [P, 1], f32)
            b2c_sb = load_const(b2c_d, [P, 1], f32)
            blbc_sb = load_const(blbc_d, [P, F_OUT], f32)
            ident_sb = load_const(ident_d, [P, P], gdt)
            iota_sb = load_const(iota_d, [P, P], gdt)
            idx_sb = [load_const(idx_d[li], [P, max(layouts[li].idx_cols, 1)],
                                 mybir.dt.int16) for li in (0, 1)]
            dl_sb = [load_const(dl_d[li], [P, max(layouts[li].n_builds, 1)],
                                f32) for li in (0, 1)]
            cf_sb = [load_const(cf_d[li], [P, max(layouts[li].n_builds, 1)],
                                f32) for li in (0, 1)]
            sw_sb = [load_const(sw_d[li], [P, TILES_PC], f32) for li in (0, 1)]
            # own-shard H kept in SBUF for the self-loop term
            h_own = [cpool.tile([P, TILES_PC, P], gdt, tag=f"h{li}own",
                                name=f"h{li}own")
                     for li in (1, 2)]

            # ---- phase 0a: own-shard H1 (for self-loops) ----
            for t in range(TILES_PC):
                hp = hpool.tile([P, F_HID], f32, tag="hpsum")
                nc.tensor.matmul(out=hp[:], lhsT=xto_sb[:, t * P:(t + 1) * P],
                                 rhs=w1_sb[:], start=True, stop=True)
                nc.scalar.activation(out=h_own[0][:, t, :], in_=hp[:],
                                     func=copyf)

            # ---- phase 0b: full H1 (replicated), streamed + bulk stores ----
            for ch in range(XCH):
                xts = xpool.tile([P, XCH_T * P], gdt, tag="xts")
                nc.sync.dma_start(
                    xts[:], xt_d[:, ch * XCH_T * P:(ch + 1) * XCH_T * P])
                hst = stpool.tile([P, XCH_T, P], gdt, tag="hst")
                for k in range(XCH_T):
                    hp = hpool.tile([P, F_HID], f32, tag="hpsum")
                    nc.tensor.matmul(out=hp[:], lhsT=xts[:, k * P:(k + 1) * P],
                                     rhs=w1_sb[:], start=True, stop=True)
                    nc.scalar.activation(out=hst[:, k, :], in_=hp[:],
                                         func=copyf)
                rows = slice(ch * XCH_T * P, (ch + 1) * XCH_T * P)
                nc.sync.dma_start(
                    h1_sec[rows, :].rearrange("(k p) f -> p k f", p=P),
                    hst[:])

            # ---- gather preps + triggers + aggregation per layer ----
            gb_tiles = [{}, {}]

            def emit_preps(li, g):
                lay = layouts[li]
                for s in range(N_SEC):
                    r = lay.run_by_gs[(g, s)]
                    if r.nblk == 0:
                        continue
                    gb = gpool.tile([P, max_run_nb, P], gdt, tag="gb")
                    ni = r.nblk * P
                    if li == 0:
                        src_ap = h1_sec[r.s * SEC_L1:(r.s + 1) * SEC_L1, :]
                    else:
                        src_ap = h2_sec[r.s][:]
                    if PREP_TRIGGER:
                        nc.gpsimd.dma_gather(
                            gb[:, :r.nblk, :], src_ap,
                            idx_sb[li][:, r.col0:r.col0 + ni // 16],
                            ni, ni, P, single_packet=False,
                            prepare_only=True,
                            sem=glane[prep_ctr[0] % N_SWDGE_LANES],
                            queue_num=r.s)
                        prep_ctr[0] += 1
                    else:
                        nc.gpsimd.dma_gather(
                            gb[:, :r.nblk, :], src_ap,
                            idx_sb[li][:, r.col0:r.col0 + ni // 16],
                            ni, ni, P, single_packet=False,
                            queue_num=r.s)
                    gb_tiles[li][r.rid] = gb

            for li in (0, 1):
                lay = layouts[li]
                if not PREP_TRIGGER and li == 1:
                    for s in range(N_SEC):
                        nc.gpsimd.collective_compute(
                            "AllGather", mybir.AluOpType.bypass,
                            replica_groups=rg,
                            ins=[h2_shard[s * SEC_L2:(s + 1) * SEC_L2, :]],
                            outs=[h2_sec[s][:]])
                for g in range(min(LOOK, N_GROUPS)):
                    emit_preps(li, g)
                # fire the first LOOK groups (layer 2: after each section's
                # AllGather)
                if PREP_TRIGGER:
                    for s in range(N_SEC):
                        if li == 1:
                            nc.gpsimd.collective_compute(
                                "AllGather", mybir.AluOpType.bypass,
                                replica_groups=rg,
                                ins=[h2_shard[s * SEC_L2:(s + 1) * SEC_L2, :]],
                                outs=[h2_sec[s][:]])
                        nc.gpsimd.trigger_dma(count=None, queue_num=s)

                bcol = b1c_sb if li == 0 else b2c_sb
                w_next = w2_sb if li == 0 else wl_sb
                n_next = F_HID if li == 0 else F_OUT
                for g in range(N_GROUPS):
                    if g + LOOK < N_GROUPS:
                        emit_preps(li, g + LOOK)
                        if PREP_TRIGGER:
                            for s in range(N_SEC):
                                nc.gpsimd.trigger_dma(count=None, queue_num=s)
                    t_lo, t_hi = GROUP_BOUNDS[g], GROUP_BOUNDS[g + 1]
                    aggp = None
                    for k, t in enumerate(range(t_lo, t_hi)):
                        if k % 4 == 0:
                            aggp = aggpool.tile([P, 512], f32, tag="aggp")
                        sl = slice((k % 4) * P, (k % 4) * P + P)
                        first = True
                        for (rid, bb, jj) in lay.tile_sched[t]:
                            mt = mpool.tile([P, P], gdt, tag="mt")
                            nc.vector.tensor_scalar(
                                out=mt[:], in0=iota_sb[:],
                                scalar1=dl_sb[li][:, jj:jj + 1],
                                scalar2=cf_sb[li][:, jj:jj + 1],
                                op0=mybir.AluOpType.is_equal,
                                op1=mybir.AluOpType.mult)
                            gb = gb_tiles[li][rid]
                            nc.tensor.matmul(out=aggp[:, sl],
                                             lhsT=gb[:, bb, :], rhs=mt[:],
                                             start=first, stop=False)
                            first = False
                        gss = spool.tile([P, P], gdt, tag="gselfs")
                        nc.scalar.activation(out=gss[:], in_=h_own[li][:, t, :],
                                             func=copyf,
                                             scale=sw_sb[li][:, t:t + 1])
                        nc.tensor.matmul(out=aggp[:, sl], lhsT=gss[:],
                                         rhs=ident_sb[:], start=first,
                                         stop=True)
                        outT = opool.tile([P, P], gdt, tag="outT")
                        nc.scalar.activation(out=outT[:], in_=aggp[:, sl],
                                             func=relu, bias=bcol[:])
                        hp2 = hpool.tile([P, n_next], f32, tag="hpsum")
                        nc.tensor.matmul(out=hp2[:], lhsT=outT[:],
                                         rhs=w_next[:], start=True, stop=True)
                        rows = slice(t * P, (t + 1) * P)
                        if li == 0:
                            nc.scalar.activation(out=h_own[1][:, t, :],
                                                 in_=hp2[:], func=copyf)
                            hsb = opool.tile([P, n_next], gdt, tag="hsb")
                            nc.scalar.activation(out=hsb[:], in_=hp2[:],
                                                 func=copyf)
                            nc.sync.dma_start(h2_shard[rows, :], hsb[:])
                        else:
                            osb = opool.tile([P, F_OUT], f32, tag="osb")
                            nc.vector.tensor_tensor(
                                out=osb[:], in0=hp2[:], in1=blbc_sb[:],
                                op=mybir.AluOpType.add)
                            nc.sync.dma_start(out_d[rows, :], osb[:])

    _patch_swdge_waits(nc)
    nc.compile()
    return nc


def _patch_swdge_waits(nc):
    """Retarget DMASW-lane waits to the per-lane gather-completion sems.

    Tile pre-bumps its DMASW lane sems at prep time for gen_mode==1 SWDGE
    preps, so consumers of the gathered data would run before the triggered
    DMA lands.  The real completion bump is the prep's on_update[0] sem
    (baked into the descriptors).  Fix each prep's on_update[0] to the
    glane sem of its assigned DMASW lane, then retarget every DMASW wait
    (consumers + drains) to that sem; targets stay valid because both count
    +16 per prep in the lane's FIFO order.
    """
    idx_to_proc = {v: k for k, v in PROC_NAME_TO_IDX.items()}
    insts = [i for blk in nc.m.functions[0].blocks for i in blk.instructions]
    sem_ids = {}
    for inst in insts:
        si = inst.sync_info
        if si is None:
            continue
        for u in si.on_update:
            if u.ant_name and u.ant_name.startswith("glane"):
                sem_ids[u.ant_name] = u.id
    for inst in insts:
        if type(inst).__name__ == "InstDMAGatherAnt" and inst.gen_mode == 1:
            lane = idx_to_proc[inst.bass_scheduled_proc]
            assert lane.startswith("DMASW"), lane
            nm = f"glane{lane[5:]}"
            u0 = inst.sync_info.on_update[0]
            u0.id = sem_ids[nm]
            u0.ant_name = nm
    for inst in insts:
        si = inst.sync_info
        if si is None:
            continue
        for w in si.on_wait:
            if w.ant_name and w.ant_name.startswith("DMASW"):
                nm = f"glane{w.ant_name.split('_')[0][5:]}"
                if nm in sem_ids:
                    w.id = sem_ids[nm]
                    w.ant_name = nm


# ---------------------------------------------------------------------------
# Entry point
# ---------------------------------------------------------------------------

def _run(x, edge_index, mask1, mask2, W1, b1, W2, b2, Wl, bl, n, n_pad):
    layouts, in_maps = _prepare(x, edge_index, mask1, mask2,
                                W1, b1, W2, b2, Wl, bl, n, n_pad)
    nc = _build(layouts, n_pad)
    res = run_bass_kernel_spmd(nc, in_maps, core_ids=list(range(N_CORES)))
    out = np.concatenate([res.results[c]["out"] for c in range(N_CORES)],
                         axis=0)
    return out[:n].astype(np.float32)


def kernel(x, edge_index, mask1, mask2, W1, b1, W2, b2, Wl, bl):
    return _run(x, edge_index, mask1, mask2, W1, b1, W2, b2, Wl, bl,
                N_NODES, N_PAD)
